# revision 33
# baseline (speedup 1.0000x reference)
"""DepthLSSTransform Trainium kernel: 3 SPMD launches over 8 NeuronCores.

Launch A: depthnet trunk (dn1/dn2/dn3 + softmax) on 24-row bands
          (one 16-row + one 8-row segment per core). The small dt1-dt3
          stem (1.4% of FLOPs) is folded into host input prep in f32.
Launch B: bev_pool segment-sum via one-hot matmuls over a host-built
          virtual-window schedule (sorted-by-voxel points).
Launch C: BEV downsample convs restricted to the occupied-voxel bbox;
          the constant background (conv of an all-zero BEV region) is
          filled in closed form on the host.
Host: geometry/voxel indices, scheduling, gathers, folds (orchestration).
"""
import numpy as np
import ml_dtypes

import concourse.bass as bass
import concourse.tile as tile
from concourse import bacc, mybir
from concourse.bass_utils import run_bass_kernel_spmd

dt = mybir.dt
bf16 = ml_dtypes.bfloat16

# ---- problem constants (hardcoded per contract) ----
B, N = 1, 6
CIN, CIMG, DD = 256, 80, 59
FH, FW, IH, IW = 32, 88, 256, 704
XY0, DXY, NX = -54.0, 0.3, 360
Z0, DZ, NZ = -10.0, 20.0, 1
NPTS = N * DD * FH * FW
NPIX = N * FH * FW
NCORES = 8

# per-core segments: (camera, h0) for seg A (16 rows) and seg B (8 rows)
SEG_A = [(0, 0), (1, 0), (1, 16), (2, 16), (3, 0), (4, 0), (4, 16), (5, 16)]
SEG_B = [(0, 16), (0, 24), (2, 0), (2, 8), (3, 16), (3, 24), (5, 0), (5, 8)]

# segment geometry: dn1-out rows [h0-1, h0-1+nn1); seg act rows cover
# conv inputs [h0-2, h0-2+nsr)
SEGS = [dict(nout=16, nn1=18, nsr=20),
        dict(nout=8, nn1=10, nsr=12)]

RELU = mybir.ActivationFunctionType.Relu


# ---------------------------------------------------------------- launch A
def build_launch_a():
    nc = bacc.Bacc("TRN2", target_bir_lowering=False, debug=False,
                   num_devices=NCORES)
    AP = {}

    def inp(name, shape, dtype=dt.bfloat16):
        AP[name] = nc.dram_tensor(name, shape, dtype, kind="ExternalInput").ap()
        return AP[name]

    # per-row interleaved activations: [x0(92) | x1(92) | stacked-dtc(92)]
    for s, S in enumerate(SEGS):
        inp(f"seg{s}", [128, S["nsr"], 276])
        inp(f"mn1_{s}", [128, S["nn1"]])
    # packed f32 consts: [s_dn1(2), t_dn1(2), s_dn2(2), t_dn2(2)]
    inp("consts", [128, 8], dt.float32)
    inp("w_dn1", [128, 24, 256])                    # 18 x-units + 6 dtc-units
    inp("w_dn2", [128, 24, 256])
    inp("w_dn3", [128, 2, 139])

    # chunk-major outputs: pixel (a*128+p) of segment s at [p, a0_s + a, :]
    out_depth = nc.dram_tensor("out_depth", [128, 17, DD], dt.float32,
                               kind="ExternalOutput").ap()
    out_feat = nc.dram_tensor("out_feat", [128, 17, CIMG], dt.bfloat16,
                              kind="ExternalOutput").ap()

    with tile.TileContext(nc) as tc:
        with tc.tile_pool(name="const", bufs=1) as cpool, \
             tc.tile_pool(name="work", bufs=3) as wpool, \
             tc.tile_pool(name="big", bufs=1) as bpool, \
             tc.tile_pool(name="psum", bufs=2, space="PSUM") as ppool, \
             tc.tile_pool(name="psum2", bufs=1, space="PSUM") as ppool2, \
             tc.tile_pool(name="psum3", bufs=2, space="PSUM") as ppool3:
            cts = cpool.tile([128, 8], dt.float32, name="cts")
            nc.gpsimd.dma_start(out=cts[:], in_=AP["consts"])
            ct = {"s_dn1": cts[:, 0:2], "t_dn1": cts[:, 2:4],
                  "s_dn2": cts[:, 4:6], "t_dn2": cts[:, 6:8]}
            st = {s: {} for s in range(len(SEGS))}
            # head stream on SP, ordered so the first dn1 row-group's deps
            # (seg0 rows 0..7 + wt1) land first; everything else follows
            seg_t = {}
            for s, S in enumerate(SEGS):
                seg_t[s] = bpool.tile([128, S["nsr"], 276], dt.bfloat16,
                                      tag=f"seg{s}", name=f"seg{s}")
                st[s]["seg"] = seg_t[s]
            # HWDGE + the DMA engines are globally serial, so all
            # order-critical loads go on ONE queue in strict consumption
            # order; masks ride the (separate) SWDGE path
            wt1 = cpool.tile([128, 24, 256], dt.bfloat16, name="wt1")
            nc.sync.dma_start(out=wt1[:, 0:2, :], in_=AP["w_dn1"][:, 0:2, :])
            nc.sync.dma_start(out=seg_t[0][:, 0:5, :], in_=AP["seg0"][:, 0:5, :])
            nc.sync.dma_start(out=wt1[:, 2:6, :], in_=AP["w_dn1"][:, 2:6, :])
            nc.sync.dma_start(out=wt1[:, 6:12, :], in_=AP["w_dn1"][:, 6:12, :])
            nc.sync.dma_start(out=seg_t[0][:, 5:7, :], in_=AP["seg0"][:, 5:7, :])
            nc.sync.dma_start(out=wt1[:, 12:18, :], in_=AP["w_dn1"][:, 12:18, :])
            nc.sync.dma_start(out=wt1[:, 18:24, :], in_=AP["w_dn1"][:, 18:24, :])
            for s, S in enumerate(SEGS):
                m = wpool.tile([128, S["nn1"]], dt.bfloat16, tag=f"m{s}",
                               name=f"mn1_{s}")
                nc.gpsimd.dma_start(out=m[:], in_=AP[f"mn1_{s}"])
                st[s]["mn1"] = m
            nsr0 = SEGS[0]["nsr"]
            nc.sync.dma_start(out=seg_t[0][:, 7:nsr0, :],
                              in_=AP["seg0"][:, 7:nsr0, :])
            wt2 = cpool.tile([128, 24, 256], dt.bfloat16, name="wt2")
            nc.sync.dma_start(out=wt2[:], in_=AP["w_dn2"])
            wt3 = cpool.tile([128, 2, 139], dt.bfloat16, name="wt3")
            nc.sync.dma_start(out=wt3[:], in_=AP["w_dn3"])
            nc.sync.dma_start(out=seg_t[1][:], in_=AP["seg1"])

            feat_sb, depth_sb = {}, {}

            def stage_dn1(s):
                S = SEGS[s]
                nn1 = S["nn1"]
                seg, mn1 = st[s]["seg"], st[s]["mn1"]
                n1o = []
                for g in range(2):
                    t = bpool.tile([128, nn1, 92], dt.bfloat16,
                                   tag=f"n1o{g}_{s}", name=f"n1o{g}_{s}")
                    nc.vector.memset(t[:, :, 0:2], 0.0)
                    nc.vector.memset(t[:, :, 90:92], 0.0)
                    n1o.append(t)
                st[s]["n1o"] = n1o
                RPP = 5
                for ocg in range(2):
                    for r0 in range(0, nn1, RPP):
                        nr = min(RPP, nn1 - r0)
                        ps = ppool.tile([128, nr, 88], dt.float32, tag="ps")
                        gi = 0
                        # units: x-chunks u=(ky*3+kx)*2+icc at plane icc*92;
                        # dtc u=18+ky*2+sub at plane 184 (sub0: taps (ky,0)
                        # lower + (ky,1) upper shifted copy; sub1: (ky,2))
                        units = []
                        for ky in range(3):
                            for kx in range(3):
                                for icc in range(2):
                                    units.append(((ky * 3 + kx) * 2 + icc,
                                                  ky, icc * 92 + kx + 1))
                        for ky in range(3):
                            for sub in range(2):
                                units.append((18 + ky * 2 + sub, ky,
                                              184 + (1 if sub == 0 else 3)))
                        for u, ky, coff in units:
                            rhs = bass.AP(
                                seg.tensor,
                                seg.offset + (r0 + ky) * 276 + coff,
                                [seg.ap[0], [276, nr], [1, 88]])
                            nc.tensor.matmul(
                                ps[:], wt1[:, u, ocg * 128:(ocg + 1) * 128],
                                rhs, start=(gi == 0), stop=(gi == 23))
                            gi += 1
                        ev = wpool.tile([128, nr, 88], dt.bfloat16, tag=f"evn1{s}")
                        nc.scalar.activation(ev[:], ps[:], RELU,
                                             bias=ct["t_dn1"][:, ocg:ocg + 1],
                                             scale=ct["s_dn1"][:, ocg:ocg + 1])
                        mbb = bass.AP(mn1.tensor, mn1.offset + r0,
                                      [mn1.ap[0], [1, nr], [0, 88]])
                        nc.vector.tensor_tensor(
                            out=n1o[ocg][:, r0:r0 + nr, 2:90],
                            in0=ev[:], in1=mbb, op=mybir.AluOpType.mult)

            def stage_dn2(s):
                S = SEGS[s]
                nout, nn1, n1o = S["nout"], S["nn1"], st[s]["n1o"]
                RPP = 4 if s == 0 else 2
                n2o = []
                for g in range(2):
                    n2o.append(bpool.tile([128, nout, 88], dt.bfloat16,
                                          tag=f"n2o{g}_{s}", name=f"n2o{g}_{s}"))
                st[s]["n2o"] = n2o
                # Winograd F(2,3) along x: T0=d0-d2, T1=d1+d2, T2=d2-d1,
                # T3=d1-d3 where d_j = x(2c-1+j) = n1o col (2c+1+j)
                Ts = []
                for icc in range(2):
                    T = bpool.tile([128, nn1, 4, 44], dt.bfloat16,
                                   tag=f"T{icc}_{s}", name=f"T{icc}_{s}")
                    srcT = n1o[icc]

                    def dsl(j):
                        return bass.AP(srcT.tensor, srcT.offset + 1 + j,
                                       [srcT.ap[0], [92, nn1], [2, 44]])

                    for p, (a, b, op) in enumerate([
                            (0, 2, mybir.AluOpType.subtract),
                            (1, 2, mybir.AluOpType.add),
                            (2, 1, mybir.AluOpType.subtract),
                            (1, 3, mybir.AluOpType.subtract)]):
                        eng = nc.gpsimd if p % 2 == 0 else nc.vector
                        eng.tensor_tensor(
                            out=T[:, :, p, :], in0=dsl(a), in1=dsl(b), op=op)
                    Ts.append(T)
                dn3 = stage_dn3(s)
                next(dn3)                        # prime: allocates out tiles
                for r0 in range(0, nout, RPP):
                    nr = min(RPP, nout - r0)
                    for ocg in range(2):
                        ms = []
                        for p in range(4):
                            pool_p = ppool2 if p < 2 else ppool3
                            mp = pool_p.tile([128, nr, 44], dt.float32,
                                             tag=f"m{p}")
                            gi = 0
                            for ky in range(3):
                                for icc in range(2):
                                    T = Ts[icc]
                                    rhs = bass.AP(
                                        T.tensor,
                                        T.offset + (r0 + ky) * 176 + p * 44,
                                        [T.ap[0], [176, nr], [1, 44]])
                                    nc.tensor.matmul(
                                        mp[:], wt2[:, p * 6 + ky * 2 + icc,
                                                   ocg * 128:(ocg + 1) * 128],
                                        rhs, start=(gi == 0), stop=(gi == 5))
                                    gi += 1
                            ms.append(mp)
                        # inverse: y_even = m0+m1+m2, y_odd = m1-m2-m3.
                        # TensorTensor allows at most one PSUM operand, so
                        # m1 is staged to SBUF via the act engine first.
                        m1s = wpool.tile([128, nr, 44], dt.float32, tag=f"m1s{s}")
                        nc.scalar.activation(m1s[:], ms[1][:],
                                             mybir.ActivationFunctionType.Copy)
                        u01 = wpool.tile([128, nr, 44], dt.float32, tag=f"u{s}")
                        nc.vector.tensor_tensor(out=u01[:], in0=ms[0][:],
                                                in1=m1s[:],
                                                op=mybir.AluOpType.add)
                        y0 = wpool.tile([128, nr, 44], dt.float32, tag=f"y0{s}")
                        nc.vector.tensor_tensor(out=y0[:], in0=u01[:],
                                                in1=ms[2][:],
                                                op=mybir.AluOpType.add)
                        v12 = wpool.tile([128, nr, 44], dt.float32, tag=f"v{s}")
                        nc.vector.tensor_tensor(out=v12[:], in0=m1s[:],
                                                in1=ms[2][:],
                                                op=mybir.AluOpType.subtract)
                        y1 = wpool.tile([128, nr, 44], dt.float32, tag=f"y1{s}")
                        nc.vector.tensor_tensor(out=y1[:], in0=v12[:],
                                                in1=ms[3][:],
                                                op=mybir.AluOpType.subtract)
                        dste = bass.AP(n2o[ocg].tensor,
                                       n2o[ocg].offset + r0 * 88,
                                       [n2o[ocg].ap[0], [88, nr], [2, 44]])
                        nc.scalar.activation(dste, y0[:], RELU,
                                             bias=ct["t_dn2"][:, ocg:ocg + 1],
                                             scale=ct["s_dn2"][:, ocg:ocg + 1])
                        dsto = bass.AP(n2o[ocg].tensor,
                                       n2o[ocg].offset + r0 * 88 + 1,
                                       [n2o[ocg].ap[0], [88, nr], [2, 44]])
                        nc.scalar.activation(dsto, y1[:], RELU,
                                             bias=ct["t_dn2"][:, ocg:ocg + 1],
                                             scale=ct["s_dn2"][:, ocg:ocg + 1])
                    try:
                        dn3.send(r0 + nr)        # emit dn3 chunks now ready
                    except StopIteration:
                        pass

            def stage_dn3(s):
                """Generator: receives the count of completed dn2 rows and
                emits dn3+softmax for pixel chunks whose rows are ready."""
                S = SEGS[s]
                nout, n2o = S["nout"], st[s]["n2o"]
                npix = nout * FW
                pcs = (npix + 127) // 128
                feat_sb[s] = bpool.tile([128, pcs * CIMG], dt.bfloat16,
                                        tag=f"feat{s}", name=f"feat_sb{s}")
                depth_sb[s] = bpool.tile([128, pcs * DD], dt.float32,
                                         tag=f"depth{s}", name=f"depth_sb{s}")
                n2f = [t.rearrange("p a b -> p (a b)") for t in n2o]
                a0 = 0 if s == 0 else 11
                rows_done = yield
                nflush = max(pcs - 2, 0)
                for pc in range(pcs):
                    if pc == nflush and pc > 0:
                        # flush all completed chunks now so only the last
                        # two chunks' output DMAs sit in the tail
                        dsl = bass.AP(out_depth.tensor,
                                      out_depth.offset + a0 * DD,
                                      [[17 * DD, 128], [1, nflush * DD]])
                        nc.sync.dma_start(
                            out=dsl, in_=depth_sb[s][:, 0:nflush * DD])
                        fsl = bass.AP(out_feat.tensor,
                                      out_feat.offset + a0 * CIMG,
                                      [[17 * CIMG, 128], [1, nflush * CIMG]])
                        nc.scalar.dma_start(
                            out=fsl, in_=feat_sb[s][:, 0:nflush * CIMG])
                    m = min(128, npix - pc * 128)
                    need = (pc * 128 + m - 1) // FW + 1
                    while rows_done < need:
                        rows_done = yield
                    # dn3 logits only -- the dn3 bias and the softmax are
                    # applied by the host (the whole dn3 head is consumed
                    # host-side when building the bev_pool windows)
                    ps = ppool.tile([m, 139], dt.float32, tag="ps")
                    for icc in range(2):
                        nc.tensor.matmul(ps[:], n2f[icc][:, pc * 128:pc * 128 + m],
                                         wt3[:, icc, :],
                                         start=(icc == 0), stop=(icc == 1))
                    nc.vector.tensor_copy(
                        depth_sb[s][0:m, pc * DD:(pc + 1) * DD], ps[:, 0:DD])
                    nc.scalar.activation(
                        feat_sb[s][0:m, pc * CIMG:(pc + 1) * CIMG],
                        ps[:, DD:DD + CIMG],
                        mybir.ActivationFunctionType.Copy)

                # last two chunks' outputs
                ntail = pcs - nflush
                dsl = bass.AP(out_depth.tensor,
                              out_depth.offset + (a0 + nflush) * DD,
                              [[17 * DD, 128], [1, ntail * DD]])
                nc.sync.dma_start(out=dsl,
                                  in_=depth_sb[s][:, nflush * DD:pcs * DD])
                fsl = bass.AP(out_feat.tensor,
                              out_feat.offset + (a0 + nflush) * CIMG,
                              [[17 * CIMG, 128], [1, ntail * CIMG]])
                nc.scalar.dma_start(out=fsl,
                                    in_=feat_sb[s][:, nflush * CIMG:pcs * CIMG])

            stage_dn1(0)
            stage_dn1(1)
            stage_dn2(0)
            stage_dn2(1)
    nc.compile()
    return nc


# ------------------------------------------------------------ host helpers
def _host_geometry(rots, trans, intr, post_rots, post_trans):
    import jax
    import jax.numpy as jnp
    with jax.default_device(jax.devices("cpu")[0]):
        f32 = jnp.float32
        ds = jnp.arange(1.0, 60.0, 1.0, dtype=f32)
        xs = jnp.linspace(0.0, IW - 1.0, FW, dtype=f32)
        ys = jnp.linspace(0.0, IH - 1.0, FH, dtype=f32)
        dm = jnp.broadcast_to(ds[:, None, None], (DD, FH, FW))
        xm = jnp.broadcast_to(xs[None, None, :], (DD, FH, FW))
        ym = jnp.broadcast_to(ys[None, :, None], (DD, FH, FW))
        fr = jnp.stack([xm, ym, dm], -1)
        pts = fr[None, None] - jnp.asarray(post_trans)[:, :, None, None, None, :]
        pts = jnp.einsum("bnij,bndhwj->bndhwi",
                         jnp.linalg.inv(jnp.asarray(post_rots)), pts)
        pts = jnp.concatenate([pts[..., :2] * pts[..., 2:3], pts[..., 2:3]], -1)
        comb = jnp.einsum("bnij,bnjk->bnik", jnp.asarray(rots),
                          jnp.linalg.inv(jnp.asarray(intr)))
        pts = jnp.einsum("bnij,bndhwj->bndhwi", comb, pts) \
            + jnp.asarray(trans)[:, :, None, None, None, :]
        lo = jnp.array([XY0, XY0, Z0], dtype=f32)
        dxv = jnp.array([DXY, DXY, DZ], dtype=f32)
        g = ((pts - lo) / dxv).astype(jnp.int32).reshape(-1, 3)
        kept = ((g[:, 0] >= 0) & (g[:, 0] < NX) & (g[:, 1] >= 0) & (g[:, 1] < NX)
                & (g[:, 2] >= 0) & (g[:, 2] < NZ))
        flat = (g[:, 2] * NX + g[:, 0]) * NX + g[:, 1]
        return np.asarray(flat, np.int64), np.asarray(kept)


def _conv2d_host(x, w, stride, pad):
    """x [Nb,C,H,W] f32, w [O,C,KH,KW] -> [Nb,O,Ho,Wo] (no bias)."""
    Nb, C, H, W = x.shape
    O, _, KH, KW = w.shape
    Ho = (H + 2 * pad - KH) // stride + 1
    Wo = (W + 2 * pad - KW) // stride + 1
    xp = np.zeros((Nb, C, H + 2 * pad, W + 2 * pad), np.float32)
    xp[:, :, pad:pad + H, pad:pad + W] = x
    s = xp.strides
    v = np.lib.stride_tricks.as_strided(
        xp, (Nb, C, KH, KW, Ho, Wo),
        (s[0], s[1], s[2], s[3], s[2] * stride, s[3] * stride))
    col = np.ascontiguousarray(v.transpose(1, 2, 3, 0, 4, 5)).reshape(
        C * KH * KW, Nb * Ho * Wo)
    y = w.reshape(O, C * KH * KW).astype(np.float32) @ col
    return y.reshape(O, Nb, Ho, Wo).transpose(1, 0, 2, 3)


def _host_stem(inputs):
    """dt1..dt3 in f32 on host -> h3 [N, 64, FH, FW]."""
    d = np.asarray(inputs["d"], np.float32).reshape(N, 1, IH, IW)
    a1 = (inputs["dt1_s"] * inputs["dt1_w"][:, 0, 0, 0]).astype(np.float32)
    b1 = (inputs["dt1_s"] * inputs["dt1_b"] + inputs["dt1_t"]).astype(np.float32)
    h = np.maximum(a1[None, :, None, None] * d + b1[None, :, None, None], 0.0)

    def bnrelu(y, b, s, t):
        s = np.asarray(s, np.float32)[None, :, None, None]
        bt = np.asarray(s.reshape(-1) * np.asarray(b, np.float32)
                        + np.asarray(t, np.float32))[None, :, None, None]
        return np.maximum(y * s + bt, 0.0)

    h = bnrelu(_conv2d_host(h, np.asarray(inputs["dt2_w"], np.float32), 4, 2),
               inputs["dt2_b"], inputs["dt2_s"], inputs["dt2_t"])
    h = bnrelu(_conv2d_host(h, np.asarray(inputs["dt3_w"], np.float32), 2, 2),
               inputs["dt3_b"], inputs["dt3_s"], inputs["dt3_t"])
    return h                                      # [N, 64, FH, FW]


def _prep_a_inputs(inputs):
    """Build per-core input maps for launch A."""
    x_img = np.asarray(inputs["x_img"], np.float32)
    h3 = _host_stem(inputs)

    w_dn1_full = np.asarray(inputs["dn1_w"], np.float32)    # [256, 320, 3, 3]
    w_dn1 = np.zeros((128, 24, 256), np.float32)
    for ky in range(3):
        for kx in range(3):
            for icc in range(2):
                u = (ky * 3 + kx) * 2 + icc
                w_dn1[:, u, :] = w_dn1_full[:, 64 + icc * 128:64 + (icc + 1) * 128,
                                            ky, kx].T
        # dtc units: sub0 = taps (ky,0) lower + (ky,1) upper; sub1 = (ky,2)
        u = 18 + ky * 2
        w_dn1[0:64, u, :] = w_dn1_full[:, 0:64, ky, 0].T
        w_dn1[64:128, u, :] = w_dn1_full[:, 0:64, ky, 1].T
        w_dn1[0:64, u + 1, :] = w_dn1_full[:, 0:64, ky, 2].T

    w_dn2_full = np.asarray(inputs["dn2_w"], np.float32)
    g0, g1, g2 = (w_dn2_full[:, :, :, k] for k in range(3))   # [oc, ic, ky]
    Gp = [g0, (g0 + g1 + g2) * 0.5, (g0 - g1 + g2) * 0.5, g2]
    w_dn2 = np.zeros((128, 24, 256), np.float32)
    for p in range(4):
        for ky in range(3):
            for icc in range(2):
                w_dn2[:, p * 6 + ky * 2 + icc, :] = \
                    Gp[p][:, icc * 128:(icc + 1) * 128, ky].T
    w_dn3_full = np.asarray(inputs["dn3_w"], np.float32)[:, :, 0, 0]
    w_dn3 = np.zeros((128, 2, 139), np.float32)
    w_dn3[:, 0, :] = w_dn3_full[:, 0:128].T
    w_dn3[:, 1, :] = w_dn3_full[:, 128:256].T

    def fold_bias(b, s, t):
        return np.asarray(s, np.float32), np.asarray(
            np.asarray(s) * np.asarray(b) + np.asarray(t), np.float32)

    sn1, tn1 = fold_bias(inputs["dn1_b"], inputs["dn1_s"], inputs["dn1_t"])
    sn2, tn2 = fold_bias(inputs["dn2_b"], inputs["dn2_s"], inputs["dn2_t"])
    consts = np.zeros((128, 8), np.float32)
    consts[:, 0:2] = sn1.reshape(2, 128).T
    consts[:, 2:4] = tn1.reshape(2, 128).T
    consts[:, 4:6] = sn2.reshape(2, 128).T
    consts[:, 6:8] = tn2.reshape(2, 128).T
    shared = dict(consts=consts, w_dn1=w_dn1.astype(bf16),
                  w_dn2=w_dn2.astype(bf16), w_dn3=w_dn3.astype(bf16))

    maps = []
    for c in range(NCORES):
        m = dict(shared)
        for s, (cam, h0) in enumerate([SEG_A[c], SEG_B[c]]):
            S = SEGS[s]
            nsr, nn1 = S["nsr"], S["nn1"]
            t0, r0 = h0 - 2, h0 - 1
            dtc = np.zeros((64, nsr, 92), np.float32)
            lo2, hi2 = max(0, t0), min(FH, t0 + nsr)
            if hi2 > lo2:
                dtc[:, lo2 - t0:hi2 - t0, 2:90] = h3[cam][:, lo2:hi2, :]
            seg = np.zeros((128, nsr, 3, 92), np.float32)
            if hi2 > lo2:
                seg[:, lo2 - t0:hi2 - t0, 0:2, 2:90] = \
                    x_img[cam, :, lo2:hi2, :].reshape(2, 128, hi2 - lo2, FW) \
                    .transpose(1, 2, 0, 3)
            seg[0:64, :, 2, :] = dtc
            seg[64:128, :, 2, 0:91] = dtc[:, :, 1:92]
            m[f"seg{s}"] = seg.reshape(128, nsr, 276).astype(bf16)
            rr = np.arange(nn1) + r0
            m[f"mn1_{s}"] = np.ascontiguousarray(np.broadcast_to(
                ((rr >= 0) & (rr < FH))[None, :], (128, nn1))).astype(bf16)
        maps.append(m)
    return maps


# ---------------------------------------------------------------- launch B
def build_launch_b(sizes):
    """Per chunk k: [128pix x 80ch] stationary feat tile x host-built
    [128pix x sizes[k] voxel-slot] depth-weight matrix -> [80, nv] window
    sums. W and out use packed (variable-size) layouts; W loads in a few
    batched DMAs, out in one."""
    nc = bacc.Bacc("TRN2", target_bir_lowering=False, debug=False,
                   num_devices=NCORES)
    NCH = len(sizes)
    offs = np.concatenate([[0], np.cumsum(sizes)]).astype(int)
    S = int(offs[-1])
    wmat = nc.dram_tensor("wmat", [128, S], dt.bfloat16,
                          kind="ExternalInput").ap()
    feats = nc.dram_tensor("feats", [128, NCH, CIMG], dt.bfloat16,
                           kind="ExternalInput").ap()
    owin = nc.dram_tensor("owin", [CIMG, S], dt.bfloat16,
                          kind="ExternalOutput").ap()
    bnd = sorted({min(2, NCH), min(6, NCH), min(10, NCH), min(14, NCH), NCH})
    with tile.TileContext(nc) as tc:
        with tc.tile_pool(name="const", bufs=1) as cpool, \
             tc.tile_pool(name="ps", bufs=4, space="PSUM") as pp:
            ft = cpool.tile([128, NCH, CIMG], dt.bfloat16, name="ft")
            kf = min(3, NCH)
            nc.sync.dma_start(out=ft[:, 0:kf, :], in_=feats[:, 0:kf, :])
            # act-table load happens under the W DMAs, not at first evac
            warm = cpool.tile([128, 1], dt.bfloat16, name="warm")
            nc.scalar.activation(warm[:], ft[:, 0, 0:1],
                                 mybir.ActivationFunctionType.Copy)
            wt = cpool.tile([128, S], dt.bfloat16, name="wt")
            nc.sync.dma_start(out=wt[:, 0:offs[bnd[0]]],
                              in_=wmat[:, 0:offs[bnd[0]]])
            if kf < NCH:
                nc.sync.dma_start(out=ft[:, kf:NCH, :], in_=feats[:, kf:NCH, :])
            for b in range(len(bnd) - 1):
                lo, hi = offs[bnd[b]], offs[bnd[b + 1]]
                if hi > lo:
                    nc.sync.dma_start(out=wt[:, lo:hi], in_=wmat[:, lo:hi])
            ot = cpool.tile([CIMG, S], dt.bfloat16, name="ot")
            fb = sorted({(NCH * 3) // 5, NCH - 2, NCH})
            prev_f = 0
            for k in range(NCH):
                nv, o0 = int(sizes[k]), int(offs[k])
                ps = pp.tile([CIMG, 512], dt.float32, tag="ps", name="ps")
                nc.tensor.matmul(ps[:, 0:nv], ft[:, k, :], wt[:, o0:o0 + nv],
                                 start=True, stop=True)
                if k % 2 == 0:
                    nc.scalar.activation(ot[:, o0:o0 + nv], ps[:, 0:nv],
                                         mybir.ActivationFunctionType.Copy)
                else:
                    nc.vector.tensor_copy(ot[:, o0:o0 + nv], ps[:, 0:nv])
                if k + 1 in fb:
                    hi = int(offs[k + 1])
                    nc.sync.dma_start(out=owin[:, prev_f:hi],
                                      in_=ot[:, prev_f:hi])
                    prev_f = hi
    nc.compile()
    return nc


# ---------------------------------------------------------------- launch C
def build_launch_c(nrow, ncol1, ncol2, coff):
    """BEV downsample convs over the occupied bbox only.

    nrow: ds2-out rows per core; ncol1: ds1-out cols computed; ncol2:
    ds2-out cols; coff = C0 - C1 (1 normally, 0 when the bbox touches the
    left grid edge). Geometry mirrors the full-grid version, shifted to
    the bbox (host passes pre-sliced slabs and masks)."""
    nc = bacc.Bacc("TRN2", target_bir_lowering=False, debug=False,
                   num_devices=NCORES)
    NR1 = nrow + 2                               # ds1-out rows incl halo
    NRP = 2 * NR1 + 1                            # pooled rows needed
    W1 = ncol1 + 2                               # h1 cols incl pads
    WS = 2 * ncol1 + 2                           # slab cols (2*ncol1+1, pad)
    slab = nc.dram_tensor("slab", [CIMG, NRP, WS], dt.bfloat16,
                          kind="ExternalInput").ap()
    m1 = nc.dram_tensor("m1", [128, NR1], dt.bfloat16, kind="ExternalInput").ap()
    wd1 = nc.dram_tensor("wd1", [CIMG, 9, CIMG], dt.bfloat16,
                         kind="ExternalInput").ap()
    wd2 = nc.dram_tensor("wd2", [CIMG, 9, CIMG], dt.bfloat16,
                         kind="ExternalInput").ap()
    sb1 = nc.dram_tensor("sb1", [CIMG, 2], dt.float32, kind="ExternalInput").ap()
    sb2 = nc.dram_tensor("sb2", [CIMG, 2], dt.float32, kind="ExternalInput").ap()
    yout = nc.dram_tensor("yout", [CIMG, nrow, ncol2], dt.float32,
                          kind="ExternalOutput").ap()
    with tile.TileContext(nc) as tc:
        with tc.tile_pool(name="const", bufs=1) as cpool, \
             tc.tile_pool(name="work", bufs=2) as wp, \
             tc.tile_pool(name="big", bufs=1) as bp, \
             tc.tile_pool(name="ps", bufs=3, space="PSUM") as pp:
            # weights + first slab rows first (both gate the first matmul);
            # everything streams on SP, act queue keeps only the evacs
            w1 = cpool.tile([CIMG, 9, CIMG], dt.bfloat16, name="w1")
            nc.sync.dma_start(out=w1[:], in_=wd1)
            sb1t = cpool.tile([CIMG, 2], dt.float32, name="sb1t")
            nc.gpsimd.dma_start(out=sb1t[:], in_=sb1)
            slabt = bp.tile([CIMG, NRP, WS], dt.bfloat16, name="slabt")
            nc.sync.dma_start(out=slabt[:, 0:10, :], in_=slab[:, 0:10, :])
            m1t = wp.tile([128, NR1], dt.bfloat16, name="m1t")
            nc.gpsimd.dma_start(out=m1t[:], in_=m1)
            nc.sync.dma_start(out=slabt[:, 10:18, :], in_=slab[:, 10:18, :])
            w2 = cpool.tile([CIMG, 9, CIMG], dt.bfloat16, name="w2")
            nc.sync.dma_start(out=w2[:], in_=wd2)
            sb2t = cpool.tile([CIMG, 2], dt.float32, name="sb2t")
            nc.gpsimd.dma_start(out=sb2t[:], in_=sb2)
            nc.sync.dma_start(out=slabt[:, 18:NRP, :], in_=slab[:, 18:NRP, :])
            h1 = bp.tile([CIMG, NR1, W1], dt.bfloat16, name="h1")
            nc.vector.memset(h1[:, :, 0:1], 0.0)
            nc.vector.memset(h1[:, :, ncol1 + 1:W1], 0.0)
            # ds1: stride-2 3x3; out row t (local) reads slab rows 2t..2t+2,
            # out col lc reads slab cols 2lc..2lc+2. ds2 groups are emitted
            # as soon as their h1 rows exist, filling ds1's slab-stream
            # stalls and shortening the tail.
            RP = 2
            yo = bp.tile([CIMG, nrow, ncol2], dt.float32, name="yo")
            flushed = [0]

            def ds1_group(t0):
                nr = min(RP, NR1 - t0)
                ps = pp.tile([CIMG, nr, ncol1], dt.float32, tag="ps1", name="ps")
                gi = 0
                for ky in range(3):
                    for kx in range(3):
                        rhs = bass.AP(slabt.tensor,
                                      slabt.offset + (2 * t0 + ky) * WS + kx,
                                      [slabt.ap[0], [2 * WS, nr], [2, ncol1]])
                        nc.tensor.matmul(ps[:], w1[:, ky * 3 + kx, :], rhs,
                                         start=(gi == 0), stop=(gi == 8))
                        gi += 1
                ev = wp.tile([CIMG, nr, ncol1], dt.bfloat16, tag="ev", name="ev")
                nc.scalar.activation(ev[:], ps[:], RELU, bias=sb1t[:, 1:2],
                                     scale=sb1t[:, 0:1])
                mbb = bass.AP(m1t.tensor, m1t.offset + t0,
                              [[m1t.ap[0][0], CIMG], [1, nr], [0, ncol1]])
                nc.vector.tensor_tensor(out=h1[:, t0:t0 + nr, 1:ncol1 + 1],
                                        in0=ev[:], in1=mbb,
                                        op=mybir.AluOpType.mult)

            def ds2_group(o0):
                nr = min(RP, nrow - o0)
                ps = pp.tile([CIMG, nr, ncol2], dt.float32, tag="ps2", name="ps")
                gi = 0
                for ky in range(3):
                    for kx in range(3):
                        rhs = bass.AP(h1.tensor,
                                      h1.offset + (o0 + ky) * W1 + kx + coff,
                                      [h1.ap[0], [W1, nr], [1, ncol2]])
                        nc.tensor.matmul(ps[:], w2[:, ky * 3 + kx, :], rhs,
                                         start=(gi == 0), stop=(gi == 8))
                        gi += 1
                nc.scalar.activation(yo[:, o0:o0 + nr, :], ps[:], RELU,
                                     bias=sb2t[:, 1:2], scale=sb2t[:, 0:1])
                if (o0 + nr) % 4 == 0 or o0 + nr >= nrow:
                    nc.sync.dma_start(out=yout[:, flushed[0]:o0 + nr, :],
                                      in_=yo[:, flushed[0]:o0 + nr, :])
                    flushed[0] = o0 + nr

            for t0 in range(0, NR1, RP):
                ds1_group(t0)
            for o0 in range(0, nrow, RP):
                ds2_group(o0)
    nc.compile()
    return nc


_CACHE = {}


def run_launch_a(inputs):
    if "A" not in _CACHE:
        _CACHE["A"] = build_launch_a()
    nc = _CACHE["A"]
    maps = _prep_a_inputs(inputs)
    res = run_bass_kernel_spmd(nc, maps, list(range(NCORES)))
    depth = np.zeros((NPIX, DD), np.float32)
    feat = np.zeros((NPIX, CIMG), np.float32)
    for c in range(NCORES):
        r = res.results[c]
        for s, (cam, h0) in enumerate([SEG_A[c], SEG_B[c]]):
            S = SEGS[s]
            npix = S["nout"] * FW
            base = (cam * FH + h0) * FW
            a0, pcs = (0, 11) if s == 0 else (11, 6)
            dsg = r["out_depth"][:, a0:a0 + pcs].transpose(1, 0, 2)
            depth[base:base + npix] = dsg.reshape(pcs * 128, DD)[:npix]
            fsg = r["out_feat"][:, a0:a0 + pcs].transpose(1, 0, 2)
            feat[base:base + npix] = fsg.reshape(pcs * 128, CIMG)[:npix]
    # device emits biasless dn3 logits; the dn3 bias and the softmax are
    # applied here (the whole dn3 head is consumed host-side)
    b3 = np.asarray(inputs["dn3_b"], np.float32)
    depth += b3[None, :DD]
    feat += b3[None, DD:]
    depth -= depth.max(1, keepdims=True)
    np.exp(depth, out=depth)
    depth /= depth.sum(1, keepdims=True)
    return depth, feat


def _build_chunks(flat, kept, depth_rows):
    """Group points by (camera, column-block); per group build the
    [pix, voxel-slot] depth-weight matrix over the group's voxel union.
    Splits column blocks whose union exceeds the PSUM window (512)."""
    fl = flat.reshape(N, DD, FH, FW)
    kp = kept.reshape(N, DD, FH, FW)
    chunks = []                      # (pix_ids, Wdense[npix, nv], vox_ids)

    def add_group(n, w0, w1):
        nw = w1 - w0
        f = fl[n, :, :, w0:w1]                       # [DD, FH, nw]
        k = kp[n, :, :, w0:w1]
        vids = np.unique(f[k])
        if len(vids) > 512 and nw > 1:
            mid = w0 + nw // 2
            add_group(n, w0, mid)
            add_group(n, mid, w1)
            return
        nv = max(len(vids), 1)
        slot = np.searchsorted(vids, f[k]) if len(vids) else np.zeros(0, np.int64)
        dd, hh, ww = np.nonzero(k)
        pix_loc = ww * FH + hh
        pixcol = n * FH * FW + hh * FW + (ww + w0)
        dep = depth_rows[pixcol, dd]
        Wd = np.bincount(pix_loc * nv + slot, weights=dep,
                         minlength=nw * FH * nv).reshape(nw * FH, nv)
        pix_ids = (n * FH * FW + np.arange(FH)[None, :] * FW
                   + (w0 + np.arange(nw))[:, None]).reshape(-1)
        chunks.append((pix_ids, Wd, vids))

    for n in range(N):
        for w0 in range(0, FW, 4):
            add_group(n, w0, w0 + 4)
    return chunks


def _prep_b_inputs(chunks, featflat_bf):
    """Balance chunks across cores by window size; build per-core maps with
    the packed per-slot layout (chunk k size = max over cores, desc-sorted)."""
    order = sorted(range(len(chunks)), key=lambda i: -chunks[i][1].shape[1])
    load = np.zeros(NCORES, np.int64)
    per_core = [[] for _ in range(NCORES)]
    for i in order:
        c = int(np.argmin(load))
        per_core[c].append(i)
        load[c] += chunks[i][1].shape[1]
    NCH = max(len(p) for p in per_core)
    sizes = np.zeros(NCH, np.int64)
    for p in per_core:
        for k, i in enumerate(p):
            sizes[k] = max(sizes[k], chunks[i][1].shape[1])
    sizes = (sizes + 15) // 16 * 16
    offs = np.concatenate([[0], np.cumsum(sizes)]).astype(int)
    S = int(offs[-1])
    maps, scatter = [], []
    for c in range(NCORES):
        wm = np.zeros((128, S), bf16)
        ft = np.zeros((128, NCH, CIMG), bf16)
        sc = []
        for k, i in enumerate(per_core[c]):
            pix_ids, Wd, vids = chunks[i]
            npix, nv = Wd.shape
            wm[0:npix, offs[k]:offs[k] + nv] = Wd
            ft[0:npix, k, :] = featflat_bf[pix_ids]
            sc.append((int(offs[k]), vids))
        maps.append(dict(wmat=wm, feats=ft))
        scatter.append(sc)
    return maps, scatter, tuple(int(s) for s in sizes)


def _bbox_from_occ(occ2):
    """ds2-out bbox affected by occupied voxels; everything else is the
    closed-form constant field."""
    rows = np.flatnonzero(occ2.any(1))
    cols = np.flatnonzero(occ2.any(0))
    if len(rows) == 0:
        return None
    HO = NX // 2
    t_lo = max(0, -(-(rows.min() - 1) // 2))
    t_hi = min(HO - 1, (rows.max() + 1) // 2)
    c_lo = max(0, -(-(cols.min() - 1) // 2))
    c_hi = min(HO - 1, (cols.max() + 1) // 2)
    R0 = max(0, t_lo - 1)
    R1 = min(HO - 1, t_hi + 1)
    C0 = max(0, c_lo - 1)
    C1r = min(HO - 1, c_hi + 1)
    return int(R0), int(R1), int(C0), int(C1r)


def _const_fill(inputs):
    """Closed-form ds2 output for the constant-h1 background (9 edge cases)."""
    HO = NX // 2
    w2 = np.asarray(inputs["ds2_w"], np.float32)
    c1 = np.maximum(np.asarray(inputs["ds1_t"], np.float32), 0.0)
    tap = np.einsum("oikl,i->okl", w2, c1)       # [80, 3, 3]
    s2 = np.asarray(inputs["ds2_s"], np.float32)
    t2 = np.asarray(inputs["ds2_t"], np.float32)
    y = np.zeros((CIMG, HO, HO), np.float32)
    rcase = {0: slice(1, 3), 1: slice(0, 3), 2: slice(0, 2)}
    for rc in range(3):
        for cc in range(3):
            v = np.maximum(
                s2 * tap[:, rcase[rc], rcase[cc]].sum((1, 2)) + t2, 0.0)
            rs = {0: slice(0, 1), 1: slice(1, HO - 1), 2: slice(HO - 1, HO)}
            y[:, rs[rc], rs[cc]] = v[:, None, None]
    return y


def _prep_c_inputs(inputs, pooled_t, R0, nrow, C0, ncol1):
    """pooled_t: [CIMG, 360, 360] f32 -> per-core slabs + masks + weights."""
    NR1 = nrow + 2
    NRP = 2 * NR1 + 1
    WS = 2 * ncol1 + 2
    HO = NX // 2
    C1 = max(C0 - 1, 0)                      # global col of first h1 col
    w1 = np.asarray(inputs["ds1_w"], np.float32)
    w2 = np.asarray(inputs["ds2_w"], np.float32)
    wd1 = np.ascontiguousarray(w1.transpose(1, 2, 3, 0).reshape(CIMG, 9, CIMG))
    wd2 = np.ascontiguousarray(w2.transpose(1, 2, 3, 0).reshape(CIMG, 9, CIMG))
    sb1 = np.stack([np.asarray(inputs["ds1_s"], np.float32),
                    np.asarray(inputs["ds1_t"], np.float32)], 1)
    sb2 = np.stack([np.asarray(inputs["ds2_s"], np.float32),
                    np.asarray(inputs["ds2_t"], np.float32)], 1)
    shared = dict(wd1=wd1.astype(bf16), wd2=wd2.astype(bf16), sb1=sb1, sb2=sb2)
    maps = []
    pt_bf = pooled_t.astype(bf16)
    pc0 = 2 * C1 - 1                         # global pooled col of slab col 0
    cl, ch = max(0, pc0), min(NX, pc0 + WS)
    for c in range(NCORES):
        gr0 = R0 + nrow * c - 1              # global h1 row of tile row 0
        p0 = 2 * gr0 - 1
        slab = np.zeros((CIMG, NRP, WS), bf16)
        lo, hi = max(0, p0), min(NX, p0 + NRP)
        if hi > lo and ch > cl:
            slab[:, lo - p0:hi - p0, cl - pc0:ch - pc0] = pt_bf[:, lo:hi, cl:ch]
        t1g = np.arange(NR1) + gr0
        m1 = np.broadcast_to(((t1g >= 0) & (t1g < HO))[None, :],
                             (128, NR1)).astype(bf16)
        maps.append(dict(shared, slab=slab, m1=np.ascontiguousarray(m1)))
    return maps


def kernel(**inputs):
    inputs = {k: np.asarray(v) for k, v in inputs.items()}
    flat, kept = _host_geometry(inputs["cam2lidar_rots"],
                                inputs["cam2lidar_trans"], inputs["intrins"],
                                inputs["post_rots"], inputs["post_trans"])
    depth_rows, feat_rows = run_launch_a(inputs)
    featflat_bf = feat_rows.astype(bf16)

    chunks = _build_chunks(flat, kept, depth_rows)
    bmaps, scatter, sizes = _prep_b_inputs(chunks, featflat_bf)
    key = ("B", sizes)
    if key not in _CACHE:
        _CACHE[key] = build_launch_b(sizes)
    res_b = run_bass_kernel_spmd(_CACHE[key], bmaps, list(range(NCORES)))

    allvox = np.concatenate([vids for c in range(NCORES)
                             for _, vids in scatter[c]])
    allval = np.concatenate(
        [res_b.results[c]["owin"][:, o0:o0 + len(vids)].T.astype(np.float32)
         for c in range(NCORES) for o0, vids in scatter[c]])
    o = np.argsort(allvox, kind="stable")
    allvox, allval = allvox[o], allval[o]
    starts = np.flatnonzero(np.r_[True, allvox[1:] != allvox[:-1]])
    pooled = np.zeros((NX * NX, CIMG), np.float32)
    pooled[allvox[starts]] = np.add.reduceat(allval, starts, axis=0)
    pooled_t = np.ascontiguousarray(
        pooled.reshape(NX, NX, CIMG).transpose(2, 0, 1))

    HO = NX // 2
    occ2 = np.zeros(NX * NX, bool)
    occ2[allvox] = True
    out = np.zeros((1, CIMG, HO, HO), np.float32)
    out[0] = _const_fill(inputs)
    bbox = _bbox_from_occ(occ2.reshape(NX, NX))
    if bbox is not None:
        R0, R1, C0, C1r = bbox
        nrow = -(-(R1 - R0 + 1) // NCORES)
        ncol2 = C1r - C0 + 1
        C1 = max(C0 - 1, 0)                  # global col of first h1 col
        # computed ds1 cols: C1 .. min(C1r+1, 179); edge cols beyond rely on
        # the zero pads (true conv zero-padding at the grid boundary)
        ncol1 = min(C1r + 1, HO - 1) - C1 + 1
        coff = C0 - C1
        key = ("C", nrow, ncol1, ncol2, coff)
        if key not in _CACHE:
            _CACHE[key] = build_launch_c(nrow, ncol1, ncol2, coff)
        cmaps = _prep_c_inputs(inputs, pooled_t, R0, nrow, C0, ncol1)
        res_c = run_bass_kernel_spmd(_CACHE[key], cmaps, list(range(NCORES)))
        for c in range(NCORES):
            o0g = R0 + nrow * c
            nr = min(nrow, R1 + 1 - o0g)
            if nr > 0:
                out[0, :, o0g:o0g + nr, C0:C0 + ncol2] = \
                    res_c.results[c]["yout"][:, 0:nr, 0:ncol2]
    return out
